# revision 10
# baseline (speedup 1.0000x reference)
"""Trainium2 Bass kernel for nn_EnhancedEncoderLayer (MQA sliding-window attention).

Strategy: sequence-parallel over S=2048 -> 8 cores x 256 rows (+halos).
Per core (all matmuls bf16 with f32 accumulate, elementwise f32):
  1. window-prediction net on its seq shard (InvRes(k3) -> gelu -> InvRes(k1)),
     local-shard mean as ratio estimate (the attention mask is a step function
     of ratio with enormous margin: any |err| < 1/29 keeps it identical, and
     local shard means agree with the global mean to ~1e-5).
  2. MQA projections in transposed [feat, seq] layout, 33-wide band attention
     computed as a dense [128q, 160k] block per 128-row query block with the
     runtime mask  allowed(d) <=> t >= 2|d|,  t = 3 + 29*ratio  (exactly
     reproduces the reference's -1e9 dense masking: exp underflows to 0).
     Softmax is max-free (scores/8 are O(1); exp cannot overflow) and batched
     across all 8 heads per query block.
  3. out-proj, SiLU gate, residual mix, RMS norm, per-core partial seq-sums.
     Batch pairs are merged into N=512 moving operands where possible.
Host side: gathers shards and subtracts the global seq-mean (a [2,512] vector
computed from the per-core partial sums).
"""
import numpy as np
import ml_dtypes

BF16 = ml_dtypes.bfloat16

B, S, D, H = 2, 2048, 512, 8
HD = D // H           # 64
NCORES = 8
SH = S // NCORES      # 256 rows per core
W = 16                # max band halfwidth (MAXW//2)
KSPAN = 160           # keys per 128-query block: 128 + 2*W
BN_S = float((1.0 + 1e-5) ** -0.5)
NEG = -1e9

_CACHE = {}


def _lhsT(w):
    # w [O, C] -> stationary-operand slab [128, C//128, O] bf16
    C = w.shape[1]
    return np.ascontiguousarray(
        w.T.reshape(C // 128, 128, w.shape[0]).transpose(1, 0, 2).astype(BF16))


def _pcol(v):
    # per-feature vector [D] -> per-partition layout [128, D//128]
    return np.ascontiguousarray(v.reshape(-1, 128).T.astype(np.float32))


def build_program():
    import concourse.bacc as bacc
    import concourse.mybir as mybir
    from concourse.tile import TileContext

    dt = mybir.dt
    f32, bf16, f32r = dt.float32, dt.bfloat16, dt.float32r
    A = mybir.AluOpType
    AF = mybir.ActivationFunctionType
    X = mybir.AxisListType.X
    XY = mybir.AxisListType.XY

    nc = bacc.Bacc("TRN2", target_bir_lowering=False, debug=False,
                   num_devices=NCORES)

    di = lambda n, s, d=f32: nc.dram_tensor(n, s, d, kind="ExternalInput")
    qT32_d = di("qT32", [B, D, SH + 2])
    qT16_d = di("qT16", [B, D, SH + 2], bf16)
    kT16_d = di("kT16", [B, D, SH + 2 * W], bf16)
    vT16_d = di("vT16", [B, D, SH + 2 * W], bf16)
    specs = {
        'wpe1T': ([128, 4, 2048], bf16), 'wpp1T': ([128, 16, 128], bf16),
        'wpe2T': ([128, 1, 512], bf16), 'wpp2T': ([128, 4, 1], bf16),
        'wqT': ([128, 4, 512], bf16), 'wkT': ([128, 4, 64], bf16),
        'wvT': ([128, 4, 64], bf16), 'woT': ([128, 4, 512], bf16),
        'wgT': ([128, 8, 512], bf16),
        'wdw1': ([128, 16, 3], f32), 'wdw2': ([128, 4], f32),
        'bq_t': ([128, 4], f32), 'bk_t': ([128, 1], f32),
        'bv2': ([128, 1], f32), 'bo_t': ([128, 4], f32),
        'bg_t': ([128, 4], f32), 'rms_t': ([128, 4], f32),
        'ident16': ([128, 128], bf16), 'ident32': ([128, 128], f32),
        'ones_k1': ([1, 128], f32), 'ones_m1': ([128, 1], f32),
        'c2band': ([128, KSPAN], f32), 'edge': ([128, 2, KSPAN], f32),
        'cst': ([1, 2], f32),
    }
    drams = {n: di(n, sh, d) for n, (sh, d) in specs.items()}

    out_d = nc.dram_tensor("out_xh", [B, SH, D], f32, kind="ExternalOutput")
    pc_d = nc.dram_tensor("pc", [128, 8], f32, kind="ExternalOutput")

    # loads issued on sync (stage-1 critical path) vs gpsimd (needed later)
    early = ['wpe1T', 'wdw1', 'wpp1T', 'wpe2T', 'wpp2T', 'wdw2', 'cst']

    with TileContext(nc) as tc:
        with tc.tile_pool(name="const", bufs=1) as cpool:
            qT16 = cpool.tile([128, 8, SH + 2], bf16, tag="qT16s")
            for b in range(B):
                for kc in range(4):
                    nc.sync.dma_start(
                        qT16[:, b * 4 + kc, :],
                        qT16_d[b, kc * 128:(kc + 1) * 128, :])
            sb = {}
            for n in early:
                sh, d = specs[n]
                sb[n] = cpool.tile(sh, d, tag=n, name=n)
                nc.sync.dma_start(sb[n][:], drams[n][:])
            for n, (sh, d) in specs.items():
                if n in early:
                    continue
                sb[n] = cpool.tile(sh, d, tag=n, name=n)
                nc.gpsimd.dma_start(sb[n][:], drams[n][:])
            (wpe1, wpp1, wpe2, wpp2, wq, wk, wv, wo, wg, wdw1, wdw2,
             bq_t, bk_t, bv2, bo_t, bg_t, rms_t, id16, id32,
             ones_k1, ones_m1, c2, edge, cst) = (
                sb['wpe1T'], sb['wpp1T'], sb['wpe2T'], sb['wpp2T'], sb['wqT'],
                sb['wkT'], sb['wvT'], sb['woT'], sb['wgT'], sb['wdw1'],
                sb['wdw2'], sb['bq_t'], sb['bk_t'], sb['bv2'], sb['bo_t'],
                sb['bg_t'], sb['rms_t'], sb['ident16'], sb['ident32'],
                sb['ones_k1'], sb['ones_m1'], sb['c2band'], sb['edge'],
                sb['cst'])

            qT32 = cpool.tile([128, 8, SH + 2], f32, tag="qT32s")
            kT16 = cpool.tile([128, 8, SH + 2 * W], bf16, tag="kT16s")
            vT16 = cpool.tile([128, 8, SH + 2 * W], bf16, tag="vT16s")
            for b in range(B):
                for kc in range(4):
                    sl = slice(kc * 128, (kc + 1) * 128)
                    nc.gpsimd.dma_start(kT16[:, b * 4 + kc, :],
                                        kT16_d[b, sl, :])
                    nc.gpsimd.dma_start(vT16[:, b * 4 + kc, :],
                                        vT16_d[b, sl, :])
                    nc.gpsimd.dma_start(qT32[:, b * 4 + kc, :],
                                        qT32_d[b, sl, :])

            # resident intermediates
            hdw = cpool.tile([128, 2, 16, SH], bf16, tag="hdw")
            g1 = cpool.tile([128, 2, SH], bf16, tag="g1")
            qp = cpool.tile([128, 2, 4, SH], bf16, tag="qp")
            kp = cpool.tile([128, 2, SH + 2 * W], bf16, tag="kp")
            vp = cpool.tile([128, 2, 3, HD], bf16, tag="vp")
            aT = cpool.tile([128, 2, 8, 128], bf16, tag="aT")
            attn32 = cpool.tile([128, 8, SH], f32, tag="attn32")
            attn16 = cpool.tile([128, 8, SH], bf16, tag="attn16")
            outr = cpool.tile([128, 8, SH], f32, tag="outr")
            xh = cpool.tile([128, 8, SH], f32, tag="xh")
            pc_sb = cpool.tile([128, 8], f32, tag="pc")
            bias2 = cpool.tile([128, 2, KSPAN], f32, tag="bias2")
            t128 = cpool.tile([128, 1], f32, tag="t128")

            # ------------- phase A: prediction net + qkv projections -------
            with tc.tile_pool(name="pa", bufs=6, space="PSUM") as pa, \
                 tc.tile_pool(name="sa", bufs=3) as sa, \
                 tc.tile_pool(name="sb1", bufs=3) as sb1:

                # inverted residual 1 (expand 512->2048, dw k3, proj -> 128)
                for mt in range(16):
                    h1 = sa.tile([128, 2, SH + 2], bf16, tag="h1")
                    for b in range(B):
                        ps = pa.tile([128, SH + 2], f32, tag="mm", bufs=4)
                        for kc in range(4):
                            nc.tensor.matmul(
                                ps[:], wpe1[:, kc, mt * 128:(mt + 1) * 128],
                                qT16[:, b * 4 + kc, :],
                                start=(kc == 0), stop=(kc == 3))
                        nc.scalar.activation(h1[:, b, :], ps[:], AF.Silu)
                    t1 = sa.tile([128, 2, SH], bf16, tag="t1")
                    nc.gpsimd.tensor_scalar_mul(
                        t1[:], h1[:, :, 2:SH + 2], wdw1[:, mt, 2:3])
                    t2 = sa.tile([128, 2, SH], bf16, tag="t2")
                    nc.vector.scalar_tensor_tensor(
                        t2[:], h1[:, :, 1:SH + 1], wdw1[:, mt, 1:2], t1[:],
                        op0=A.mult, op1=A.add)
                    t3 = sa.tile([128, 2, SH], bf16, tag="t3")
                    nc.vector.scalar_tensor_tensor(
                        t3[:], h1[:, :, 0:SH], wdw1[:, mt, 0:1], t2[:],
                        op0=A.mult, op1=A.add)
                    nc.scalar.activation(hdw[:, :, mt, :], t3[:], AF.Silu,
                                         scale=BN_S)
                # project 2048 -> 128 (both batches, N=512), * BN_S, gelu
                ps2 = pa.tile([128, 2, SH], f32, tag="mm", bufs=4)
                for kc in range(16):
                    nc.tensor.matmul(ps2[:], wpp1[:, kc, :],
                                     hdw[:, :, kc, :],
                                     start=(kc == 0), stop=(kc == 15))
                nc.scalar.activation(g1[:, :, :], ps2[:], AF.Gelu, scale=BN_S)

                # inverted residual 2 (128->512, dw k1, proj -> 1) + ratio
                ps4 = pa.tile([1, 2, SH], f32, tag="p4", bufs=1)
                for mt in range(4):
                    ps3 = pa.tile([128, 2, SH], f32, tag="mm", bufs=4)
                    nc.tensor.matmul(ps3[:],
                                     wpe2[:, 0, mt * 128:(mt + 1) * 128],
                                     g1[:, :, :], start=True, stop=True)
                    s1 = sb1.tile([128, 2, SH], bf16, tag="s1")
                    nc.scalar.activation(s1[:], ps3[:], AF.Silu)
                    s2 = sb1.tile([128, 2, SH], bf16, tag="s2")
                    nc.vector.tensor_scalar_mul(s2[:], s1[:],
                                                wdw2[:, mt:mt + 1])
                    s3 = sb1.tile([128, 2, SH], bf16, tag="s3")
                    nc.scalar.activation(s3[:], s2[:], AF.Silu, scale=BN_S)
                    nc.tensor.matmul(ps4[:], wpp2[:, mt, :], s3[:],
                                     start=(mt == 0), stop=(mt == 3))
                r0 = sb1.tile([1, 1], f32, tag="r0")
                nc.vector.reduce_sum(r0[:], ps4[:], axis=XY)
                t_sc = sb1.tile([1, 1], f32, tag="tsc")
                nc.scalar.activation(t_sc[:], r0[:], AF.Identity,
                                     scale=29.0 * BN_S / (B * SH),
                                     bias=cst[0:1, 1:2])
                ps_t = pa.tile([128, 1], f32, tag="mm", bufs=4)
                nc.tensor.matmul(ps_t[:], ones_k1[:], t_sc[:], start=True,
                                 stop=True)
                nc.scalar.copy(t128[:], ps_t[:])
                mask = sb1.tile([128, KSPAN], f32, tag="mask")
                nc.vector.tensor_scalar(mask[:], c2[:], t128[:, 0:1], None,
                                        op0=A.is_le)
                bb = sb1.tile([128, KSPAN], f32, tag="bb")
                nc.scalar.activation(bb[:], mask[:], AF.Copy, scale=1e9,
                                     bias=-1e9)
                for blk in range(2):
                    nc.vector.tensor_add(bias2[:, blk, :], bb[:],
                                         edge[:, blk, :])

                # q/k/v projections
                for mt in range(4):
                    psq = pa.tile([128, 2, SH], f32, tag="mm", bufs=4)
                    for kc in range(4):
                        nc.tensor.matmul(
                            psq[:], wq[:, kc, mt * 128:(mt + 1) * 128],
                            qT16[:, kc::4, 1:SH + 1],
                            start=(kc == 0), stop=(kc == 3))
                    nc.scalar.activation(qp[:, :, mt, :], psq[:], AF.Identity,
                                         bias=bq_t[:, mt:mt + 1])
                for b in range(B):
                    psk = pa.tile([128, SH + 2 * W], f32, tag="kpp", bufs=1)
                    for half in (0, 64):
                        for kc in range(4):
                            nc.tensor.matmul(psk[half:half + 64, :],
                                             wk[:, kc, :],
                                             kT16[:, b * 4 + kc, :],
                                             start=(kc == 0), stop=(kc == 3))
                    nc.scalar.activation(kp[:, b, :], psk[:], AF.Identity,
                                         bias=bk_t[:, 0:1])
                    for sub in range(3):
                        rows = 128 if sub < 2 else 2 * W
                        psv = pa.tile([128, HD], f32, tag="mm", bufs=4)
                        for kc in range(4):
                            nc.tensor.matmul(
                                psv[:rows, :],
                                vT16[:, b * 4 + kc,
                                     sub * 128:sub * 128 + rows],
                                wv[:, kc, :],
                                start=(kc == 0), stop=(kc == 3))
                        nc.vector.tensor_copy(vp[:rows, b, sub, :],
                                              psv[:rows, :])

            # ------------- phase B1: band attention (batched softmax) ------
            with tc.tile_pool(name="ps_s", bufs=3, space="PSUM") as ps_s, \
                 tc.tile_pool(name="ps_t", bufs=2, space="PSUM") as ps_tp, \
                 tc.tile_pool(name="ps_at", bufs=2, space="PSUM") as ps_at, \
                 tc.tile_pool(name="sat", bufs=2) as sat:
                for b in range(B):
                    for blk in range(2):
                        q0 = blk * 128
                        sc = sat.tile([128, 8, KSPAN], f32, tag="sc")
                        for h in range(H):
                            hp, ho = h // 2, (h % 2) * 64
                            pss = ps_s.tile([128, KSPAN], f32, tag="s")
                            nc.tensor.matmul(
                                pss[:],
                                qp[ho:ho + 64, b, hp, q0:q0 + 128],
                                kp[ho:ho + 64, b, q0:q0 + KSPAN],
                                start=True, stop=True)
                            nc.vector.tensor_add(sc[:, h, :], pss[:],
                                                 bias2[:, blk, :])
                        pr = sat.tile([128, 8, KSPAN], bf16, tag="pr")
                        nc.scalar.activation(pr[:], sc[:], AF.Exp,
                                             scale=0.125)
                        z = sat.tile([128, 8], f32, tag="z")
                        nc.vector.reduce_sum(z[:], pr[:], axis=X)
                        rz = sat.tile([128, 8], f32, tag="rz")
                        nc.vector.reciprocal(rz[:], z[:])
                        prn = sat.tile([128, 8, KSPAN], bf16, tag="prn")
                        nc.vector.tensor_mul(
                            prn[:], pr[:],
                            rz[:, :, None].broadcast_to([128, 8, KSPAN]))
                        pat = None
                        for h in range(H):
                            hp, ho = h // 2, (h % 2) * 64
                            pt1 = ps_tp.tile([128, 128], bf16, tag="tt")
                            nc.tensor.transpose(pt1[:], prn[:, h, 0:128],
                                                id16[:])
                            pt2 = ps_tp.tile([32, 128], bf16, tag="tt")
                            nc.tensor.transpose(pt2[:], prn[:, h, 128:KSPAN],
                                                id16[:])
                            pT1 = sat.tile([128, 128], bf16, tag="pT1")
                            nc.vector.tensor_copy(pT1[:], pt1[:])
                            pT2 = sat.tile([32, 128], bf16, tag="pT2")
                            nc.scalar.copy(pT2[:], pt2[:])
                            if h % 2 == 0:
                                pat = ps_at.tile([128, 128], f32, tag="at")
                            nc.tensor.matmul(pat[ho:ho + 64, :],
                                             vp[:, b, blk, :], pT1[:],
                                             start=True, stop=False)
                            nc.tensor.matmul(pat[ho:ho + 64, :],
                                             vp[0:2 * W, b, blk + 1, :],
                                             pT2[:], start=False, stop=True)
                            if h % 2 == 1:
                                nc.scalar.activation(
                                    aT[:, b, hp * 2 + blk, :], pat[:],
                                    AF.Identity, bias=bv2[:, 0:1])

            # ------------- phase B2: wo, gate, residual, rms ---------------
            with tc.tile_pool(name="pb2", bufs=3, space="PSUM") as pb2, \
                 tc.tile_pool(name="pbs", bufs=2, space="PSUM") as pbs, \
                 tc.tile_pool(name="pbt", bufs=2, space="PSUM") as pbt, \
                 tc.tile_pool(name="sb2", bufs=3) as sb2, \
                 tc.tile_pool(name="sxh", bufs=2) as sxh:
                for mt in range(4):
                    pwo = pb2.tile([128, 2, SH], f32, tag="mm")
                    for kc in range(4):
                        nc.tensor.matmul(
                            pwo[:], wo[:, kc, mt * 128:(mt + 1) * 128],
                            aT[:, :, kc * 2:kc * 2 + 2, :],
                            start=(kc == 0), stop=(kc == 3))
                    nc.vector.tensor_scalar_add(attn32[:, mt::4, :], pwo[:],
                                                bo_t[:, mt:mt + 1])
                    nc.scalar.activation(attn16[:, mt::4, :], pwo[:],
                                         AF.Identity,
                                         bias=bo_t[:, mt:mt + 1])
                pss = None
                for mt in range(4):
                    pg = pb2.tile([128, 2, SH], f32, tag="mm")
                    for kc in range(8):
                        rhs = (qT16[:, kc::4, 1:SH + 1] if kc < 4
                               else attn16[:, (kc - 4)::4, :])
                        nc.tensor.matmul(
                            pg[:], wg[:, kc, mt * 128:(mt + 1) * 128],
                            rhs, start=(kc == 0), stop=(kc == 7))
                    gate = sb2.tile([128, 2, SH], f32, tag="gate")
                    nc.scalar.activation(gate[:], pg[:], AF.Silu,
                                         bias=bg_t[:, mt:mt + 1])
                    d1 = sb2.tile([128, 2, SH], f32, tag="d1")
                    nc.vector.tensor_sub(d1[:], qT32[:, mt::4, 1:SH + 1],
                                         attn32[:, mt::4, :])
                    u = sb2.tile([128, 2, SH], f32, tag="u")
                    nc.vector.tensor_mul(u[:], gate[:], d1[:])
                    nc.vector.tensor_add(outr[:, mt::4, :],
                                         attn32[:, mt::4, :], u[:])
                    sq = sb2.tile([128, 2, SH], f32, tag="sq")
                    nc.scalar.square(sq[:], outr[:, mt::4, :])
                    if mt == 0:
                        pss = pbs.tile([1, 2, SH], f32, tag="ss", bufs=1)
                    nc.tensor.matmul(pss[:], ones_m1[:], sq[:],
                                     start=(mt == 0), stop=(mt == 3))
                sqr = sb2.tile([1, 2, SH], f32, tag="sqr")
                nc.scalar.activation(sqr[:], pss[:], AF.Sqrt,
                                     scale=1.0 / D, bias=cst[0:1, 0:1])
                rstd = sb2.tile([1, 2, SH], f32, tag="rstd")
                nc.vector.reciprocal(rstd[:], sqr[:])
                prb = pbs.tile([128, 2, SH], f32, tag="rb", bufs=1)
                nc.tensor.matmul(prb[:], ones_k1[:], rstd[:], start=True,
                                 stop=True)
                for mt in range(4):
                    nc.vector.scalar_tensor_tensor(
                        xh[:, mt::4, :], outr[:, mt::4, :],
                        rms_t[:, mt:mt + 1], prb[:],
                        op0=A.mult, op1=A.mult)
                for bm in range(8):
                    nc.vector.reduce_sum(pc_sb[:, bm:bm + 1],
                                         xh[:, bm, :], axis=X)
                for b in range(B):
                    for blk in range(2):
                        xt = sxh.tile([128, 512], f32, tag="xt")
                        for mt in range(4):
                            px = pbt.tile([128, 128], f32, tag="px")
                            nc.tensor.transpose(
                                px[:], xh[:, b * 4 + mt,
                                          blk * 128:blk * 128 + 128],
                                id32[:])
                            if mt % 2 == 0:
                                nc.scalar.copy(
                                    xt[:, mt * 128:(mt + 1) * 128], px[:])
                            else:
                                nc.vector.tensor_copy(
                                    xt[:, mt * 128:(mt + 1) * 128], px[:])
                        nc.sync.dma_start(
                            out_d[b, blk * 128:blk * 128 + 128, :], xt[:])
                nc.sync.dma_start(pc_d[:], pc_sb[:])

    nc.compile()
    return nc


def prep_inputs(inputs):
    """Full inputs dict -> list of 8 per-core in_maps."""
    f = lambda k: np.asarray(inputs[k], np.float32)
    query, key, value = f('query'), f('key'), f('value')

    qt = np.swapaxes(query, 1, 2)          # [B, D, S]
    kt = np.swapaxes(key, 1, 2)
    vt = np.swapaxes(value, 1, 2)
    qp1 = np.pad(qt, ((0, 0), (0, 0), (1, 1)))
    kpw = np.pad(kt, ((0, 0), (0, 0), (W, W)))
    vpw = np.pad(vt, ((0, 0), (0, 0), (W, W)))

    r = np.arange(128)
    j = np.arange(KSPAN)
    shared = {
        'wpe1T': _lhsT(f('wp_e1')), 'wpp1T': _lhsT(f('wp_p1')),
        'wpe2T': _lhsT(f('wp_e2')), 'wpp2T': _lhsT(f('wp_p2')),
        'wqT': _lhsT(f('wq')), 'wkT': _lhsT(f('wk')), 'wvT': _lhsT(f('wv')),
        'woT': _lhsT(f('wo')), 'wgT': _lhsT(f('wg')),
        'wdw1': np.ascontiguousarray(
            f('wp_dw1').reshape(16, 128, 3).transpose(1, 0, 2)),
        'wdw2': _pcol(f('wp_dw2')[:, 0]),
        'bq_t': _pcol(f('bq')),
        'bk_t': np.tile(f('bk'), 2).reshape(128, 1).astype(np.float32),
        'bv2': np.tile(f('bv'), 2).reshape(128, 1).astype(np.float32),
        'bo_t': _pcol(f('bo')), 'bg_t': _pcol(f('bg')),
        'rms_t': _pcol(f('rms_w')),
        'ident16': np.eye(128, dtype=BF16),
        'ident32': np.eye(128, dtype=np.float32),
        'ones_k1': np.ones((1, 128), np.float32),
        'ones_m1': np.ones((128, 1), np.float32),
        'c2band': (2.0 * np.abs(j[None, :] - W - r[:, None])
                   ).astype(np.float32),
        'cst': np.array([[1e-6, 3.0]], np.float32),
    }

    maps = []
    for c in range(NCORES):
        s0 = c * SH
        m = dict(shared)
        m['qT32'] = np.ascontiguousarray(qp1[:, :, s0:s0 + SH + 2])
        m['qT16'] = m['qT32'].astype(BF16)
        m['kT16'] = np.ascontiguousarray(
            kpw[:, :, s0:s0 + SH + 2 * W]).astype(BF16)
        m['vT16'] = np.ascontiguousarray(
            vpw[:, :, s0:s0 + SH + 2 * W]).astype(BF16)
        edge = np.zeros((128, 2, KSPAN), np.float32)
        for blk in range(2):
            kidx = s0 + blk * 128 - W + j         # global key index per col
            edge[:, blk, (kidx < 0) | (kidx >= S)] = NEG
        m['edge'] = edge
        maps.append(m)
    return maps


def _get_program():
    if 'nc' not in _CACHE:
        _CACHE['nc'] = build_program()
    return _CACHE['nc']


def finish(results):
    """Gather per-core outputs -> full [B, S, D] (global seq-mean subtract)."""
    xh = np.concatenate([r['out_xh'] for r in results], axis=1)
    pc = np.zeros((128, 8), np.float64)
    for r in results:
        pc += r['pc']
    # pc[p, b*4+mt] -> batch b, feature o = mt*128+p
    pcv = pc.reshape(128, 2, 4).transpose(1, 2, 0).reshape(2, 512)
    mean = (pcv / S).astype(np.float32)
    return (xh - mean[:, None, :]).astype(np.float32)


def kernel(**inputs):
    from concourse.bass_utils import run_bass_kernel_spmd
    nc = _get_program()
    maps = prep_inputs(inputs)
    res = run_bass_kernel_spmd(nc, maps, list(range(NCORES)))
    return finish(res.results)


# revision 13
# speedup vs baseline: 1.8180x; 1.8180x over previous
"""Trainium2 Bass kernel for nn_EnhancedEncoderLayer (MQA sliding-window attention).

Strategy: sequence-parallel over S=2048 -> 8 cores x 256 rows (+halos).
Per core (all matmuls bf16 with f32 accumulate, elementwise f32):
  1. window-prediction net on its seq shard (InvRes(k3) -> gelu -> InvRes(k1)),
     local-shard mean as ratio estimate (the attention mask is a step function
     of ratio with enormous margin: any |err| < 1/29 keeps it identical, and
     local shard means agree with the global mean to ~1e-5).
  2. MQA projections in transposed [feat, seq] layout, 33-wide band attention
     computed as a dense [128q, 160k] block per 128-row query block with the
     runtime mask  allowed(d) <=> t >= 2|d|,  t = 3 + 29*ratio  (exactly
     reproduces the reference's -1e9 dense masking: exp underflows to 0).
     Softmax is max-free (scores/8 are O(1); exp cannot overflow) and batched
     across all 8 heads per query block.
  3. out-proj, SiLU gate, residual mix, RMS norm, per-core partial seq-sums.
     Batch pairs are merged into contiguous N=512 moving operands.
Host side: gathers shards and subtracts the global seq-mean (a [2,512] vector
computed from the per-core partial sums).  v-bias is folded into bo on host
(bo_eff = bo + wo @ tile(bv, H)).
"""
import numpy as np
import ml_dtypes

BF16 = ml_dtypes.bfloat16

B, S, D, H = 2, 2048, 512, 8
HD = D // H           # 64
NCORES = 8
SH = S // NCORES      # 256 rows per core
W = 16                # max band halfwidth (MAXW//2)
KSPAN = 160           # keys per 128-query block: 128 + 2*W
BN_S = float((1.0 + 1e-5) ** -0.5)
NEG = -1e9

# merged weight slab layouts: name -> cols
EARLY16 = [('wpe1T', 4 * 2048), ('wpp1T', 16 * 128), ('wpe2T', 512),
           ('wpp2T', 4)]
LATE16 = [('wqT', 4 * 512), ('wkT', 4 * 64), ('wvT', 4 * 64),
          ('woT', 4 * 512), ('wgT', 8 * 512), ('ident16', 128)]
EARLY32 = [('wdw1', 48), ('wdw2', 4), ('cst', 2)]
LATE32 = [('ident32', 128), ('bq_t', 4), ('bk_t', 1), ('bo_t', 4),
          ('bg_t', 4), ('rms_t', 4), ('ones_k1', 128), ('ones_m1', 1),
          ('c2band', KSPAN), ('edge', 2 * KSPAN)]


def _cols(layout):
    return sum(c for _, c in layout)


def _off(layout, name):
    o = 0
    for n, c in layout:
        if n == name:
            return o
        o += c
    raise KeyError(name)


_CACHE = {}


def _lhsT(w):
    # w [O, C] -> stationary-operand slab [128, C//128 * O] bf16
    C = w.shape[1]
    return np.ascontiguousarray(
        w.T.reshape(C // 128, 128, w.shape[0]).transpose(1, 0, 2)
        .reshape(128, -1).astype(BF16))


def _pcol(v):
    # per-feature vector [D] -> per-partition layout [128, D//128]
    return np.ascontiguousarray(v.reshape(-1, 128).T.astype(np.float32))


def build_program():
    import concourse.bacc as bacc
    import concourse.mybir as mybir
    from concourse.tile import TileContext

    dt = mybir.dt
    f32, bf16 = dt.float32, dt.bfloat16
    A = mybir.AluOpType
    AF = mybir.ActivationFunctionType
    X = mybir.AxisListType.X
    XY = mybir.AxisListType.XY

    nc = bacc.Bacc("TRN2", target_bir_lowering=False, debug=False,
                   num_devices=NCORES)

    di = lambda n, s, d=f32: nc.dram_tensor(n, s, d, kind="ExternalInput")
    # activation tensors, host layout [128, kc*2+b, s] (kc-major, b-minor)
    qT32_d = di("qT32", [128, 8, SH + 2])
    qT16_d = di("qT16", [128, 8, SH + 2], bf16)
    kT16_d = di("kT16", [128, 8, SH + 2 * W], bf16)
    vT16_d = di("vT16", [128, 8, SH + 2 * W], bf16)
    e16_d = di("early16", [128, _cols(EARLY16)], bf16)
    l16_d = di("late16", [128, _cols(LATE16)], bf16)
    e32_d = di("early32", [128, _cols(EARLY32)])
    l32_d = di("late32", [128, _cols(LATE32)])

    out_d = nc.dram_tensor("out_xh", [B, SH, D], f32, kind="ExternalOutput")
    pc_d = nc.dram_tensor("pc", [128, 8], f32, kind="ExternalOutput")

    with TileContext(nc) as tc:
        with tc.tile_pool(name="const", bufs=1) as cpool:
            # ordered loads: stage-1 critical first, everything on sync
            qT16 = cpool.tile([128, 8, SH + 2], bf16, tag="qT16s")
            nc.sync.dma_start(qT16[:], qT16_d[:])
            e16 = cpool.tile([128, _cols(EARLY16)], bf16, tag="e16")
            nc.sync.dma_start(e16[:], e16_d[:])
            e32 = cpool.tile([128, _cols(EARLY32)], f32, tag="e32")
            nc.sync.dma_start(e32[:], e32_d[:])
            kT16 = cpool.tile([128, 8, SH + 2 * W], bf16, tag="kT16s")
            nc.sync.dma_start(kT16[:], kT16_d[:])
            vT16 = cpool.tile([128, 8, SH + 2 * W], bf16, tag="vT16s")
            nc.sync.dma_start(vT16[:], vT16_d[:])
            l16 = cpool.tile([128, _cols(LATE16)], bf16, tag="l16")
            nc.sync.dma_start(l16[:], l16_d[:])
            l32 = cpool.tile([128, _cols(LATE32)], f32, tag="l32")
            nc.sync.dma_start(l32[:], l32_d[:])
            qT32 = cpool.tile([128, 8, SH + 2], f32, tag="qT32s")
            nc.sync.dma_start(qT32[:], qT32_d[:])

            # weight slab views
            wpe1 = e16[:, 0:8192].rearrange("p (kc m) -> p kc m", kc=4)
            wpp1 = e16[:, 8192:8192 + 2048].rearrange(
                "p (kc m) -> p kc m", kc=16)
            wpe2 = e16[:, 10240:10752]
            wpp2 = e16[:, 10752:10756].rearrange("p (kc m) -> p kc m", kc=4)
            lo = lambda n: _off(LATE16, n)
            wq = l16[:, lo('wqT'):lo('wqT') + 2048].rearrange(
                "p (kc m) -> p kc m", kc=4)
            wk = l16[:, lo('wkT'):lo('wkT') + 256].rearrange(
                "p (kc m) -> p kc m", kc=4)
            wv = l16[:, lo('wvT'):lo('wvT') + 256].rearrange(
                "p (kc m) -> p kc m", kc=4)
            wo = l16[:, lo('woT'):lo('woT') + 2048].rearrange(
                "p (kc m) -> p kc m", kc=4)
            wg = l16[:, lo('wgT'):lo('wgT') + 4096].rearrange(
                "p (kc m) -> p kc m", kc=8)
            id16 = l16[:, lo('ident16'):lo('ident16') + 128]
            wdw1 = e32[:, 0:48].rearrange("p (mt t) -> p mt t", mt=16)
            wdw2 = e32[:, 48:52]
            cst = e32[0:1, 52:54]
            go = lambda n: _off(LATE32, n)
            id32 = l32[:, go('ident32'):go('ident32') + 128]
            bq_t = l32[:, go('bq_t'):go('bq_t') + 4]
            bk_t = l32[:, go('bk_t'):go('bk_t') + 1]
            bo_t = l32[:, go('bo_t'):go('bo_t') + 4]
            bg_t = l32[:, go('bg_t'):go('bg_t') + 4]
            rms_t = l32[:, go('rms_t'):go('rms_t') + 4]
            ones_k1 = l32[0:1, go('ones_k1'):go('ones_k1') + 128]
            ones_m1 = l32[:, go('ones_m1'):go('ones_m1') + 1]
            c2 = l32[:, go('c2band'):go('c2band') + KSPAN]
            edge = l32[:, go('edge'):go('edge') + 2 * KSPAN].rearrange(
                "p (blk j) -> p blk j", blk=2)

            # resident intermediates (batch-pair contiguous layouts)
            hdw = cpool.tile([128, 16, 2, SH], bf16, tag="hdw")   # (mt, b)
            g1 = cpool.tile([128, 2, SH], bf16, tag="g1")
            qp = cpool.tile([128, 4, 2, SH], bf16, tag="qp")      # (hp, b)
            kp = cpool.tile([128, 2, SH + 2 * W], bf16, tag="kp")
            vp = cpool.tile([128, 2, 3, HD], bf16, tag="vp")
            aT = cpool.tile([128, 4, 2, 2, 128], bf16, tag="aT")  # (hp,b,blk)
            attn32 = cpool.tile([128, 4, 2, SH], f32, tag="attn32")
            attn16 = cpool.tile([128, 4, 2, SH], bf16, tag="attn16")
            outr = cpool.tile([128, 4, 2, SH], f32, tag="outr")
            xh = cpool.tile([128, 4, 2, SH], f32, tag="xh")
            pc_sb = cpool.tile([128, 8], f32, tag="pc")
            bias2 = cpool.tile([128, 2, KSPAN], f32, tag="bias2")
            t128 = cpool.tile([128, 1], f32, tag="t128")

            # ------------- phase A: prediction net + qkv projections -------
            with tc.tile_pool(name="pa", bufs=6, space="PSUM") as pa, \
                 tc.tile_pool(name="sa", bufs=3) as sa, \
                 tc.tile_pool(name="sb1", bufs=3) as sb1:

                # inverted residual 1 (expand 512->2048, dw k3, proj -> 128)
                for mt in range(16):
                    for b in range(B):
                        ps = pa.tile([128, SH + 2], f32, tag="mm", bufs=4)
                        for kc in range(4):
                            nc.tensor.matmul(
                                ps[:], wpe1[:, kc, mt * 128:(mt + 1) * 128],
                                qT16[:, kc * 2 + b, :],
                                start=(kc == 0), stop=(kc == 3))
                        h1 = sa.tile([128, SH + 2], bf16, tag="h1")
                        nc.scalar.activation(h1[:], ps[:], AF.Silu)
                        t1 = sa.tile([128, SH], bf16, tag="t1")
                        nc.vector.tensor_scalar_mul(
                            t1[:], h1[:, 2:SH + 2], wdw1[:, mt, 2:3])
                        t2 = sa.tile([128, SH], bf16, tag="t2")
                        nc.vector.scalar_tensor_tensor(
                            t2[:], h1[:, 1:SH + 1], wdw1[:, mt, 1:2], t1[:],
                            op0=A.mult, op1=A.add)
                        t3 = sa.tile([128, SH], bf16, tag="t3")
                        nc.vector.scalar_tensor_tensor(
                            t3[:], h1[:, 0:SH], wdw1[:, mt, 0:1], t2[:],
                            op0=A.mult, op1=A.add)
                        nc.scalar.activation(hdw[:, mt, b, :], t3[:],
                                             AF.Silu, scale=BN_S)
                # project 2048 -> 128 (both batches, N=512), * BN_S, gelu
                ps2 = pa.tile([128, 2, SH], f32, tag="mm", bufs=4)
                for kc in range(16):
                    nc.tensor.matmul(ps2[:], wpp1[:, kc, :],
                                     hdw[:, kc, :, :],
                                     start=(kc == 0), stop=(kc == 15))
                nc.scalar.activation(g1[:], ps2[:], AF.Gelu, scale=BN_S)

                # inverted residual 2 (128->512, dw k1, proj -> 1) + ratio
                ps4 = pa.tile([1, 2, SH], f32, tag="p4", bufs=1)
                for mt in range(4):
                    ps3 = pa.tile([128, 2, SH], f32, tag="mm", bufs=4)
                    nc.tensor.matmul(ps3[:],
                                     wpe2[:, mt * 128:(mt + 1) * 128],
                                     g1[:], start=True, stop=True)
                    s1 = sb1.tile([128, 2, SH], bf16, tag="s1")
                    nc.scalar.activation(s1[:], ps3[:], AF.Silu)
                    s2 = sb1.tile([128, 2, SH], bf16, tag="s2")
                    nc.vector.tensor_scalar_mul(s2[:], s1[:],
                                                wdw2[:, mt:mt + 1])
                    s3 = sb1.tile([128, 2, SH], bf16, tag="s3")
                    nc.scalar.activation(s3[:], s2[:], AF.Silu, scale=BN_S)
                    nc.tensor.matmul(ps4[:], wpp2[:, mt, :], s3[:],
                                     start=(mt == 0), stop=(mt == 3))
                r0 = sb1.tile([1, 1], f32, tag="r0")
                nc.vector.reduce_sum(r0[:], ps4[:], axis=XY)
                t_sc = sb1.tile([1, 1], f32, tag="tsc")
                nc.scalar.activation(t_sc[:], r0[:], AF.Identity,
                                     scale=29.0 * BN_S / (B * SH),
                                     bias=cst[0:1, 1:2])
                ps_t = pa.tile([128, 1], f32, tag="mm", bufs=4)
                nc.tensor.matmul(ps_t[:], ones_k1[:], t_sc[:], start=True,
                                 stop=True)
                nc.scalar.copy(t128[:], ps_t[:])
                mask = sb1.tile([128, KSPAN], f32, tag="mask")
                nc.vector.tensor_scalar(mask[:], c2[:], t128[:, 0:1], None,
                                        op0=A.is_le)
                bb = sb1.tile([128, KSPAN], f32, tag="bb")
                nc.scalar.activation(bb[:], mask[:], AF.Copy, scale=1e9,
                                     bias=-1e9)
                for blk in range(2):
                    nc.vector.tensor_add(bias2[:, blk, :], bb[:],
                                         edge[:, blk, :])

                # q/k/v projections
                for mt in range(4):
                    psq = pa.tile([128, 2, SH], f32, tag="mm", bufs=4)
                    for kc in range(4):
                        nc.tensor.matmul(
                            psq[:], wq[:, kc, mt * 128:(mt + 1) * 128],
                            qT16[:, kc * 2:kc * 2 + 2, 1:SH + 1],
                            start=(kc == 0), stop=(kc == 3))
                    nc.scalar.activation(qp[:, mt, :, :], psq[:], AF.Identity,
                                         bias=bq_t[:, mt:mt + 1])
                for b in range(B):
                    psk = pa.tile([128, SH + 2 * W], f32, tag="kpp", bufs=1)
                    for half in (0, 64):
                        for kc in range(4):
                            nc.tensor.matmul(psk[half:half + 64, :],
                                             wk[:, kc, :],
                                             kT16[:, kc * 2 + b, :],
                                             start=(kc == 0), stop=(kc == 3))
                    nc.scalar.activation(kp[:, b, :], psk[:], AF.Identity,
                                         bias=bk_t[:, 0:1])
                    for sub in range(3):
                        rows = 128 if sub < 2 else 2 * W
                        psv = pa.tile([128, HD], f32, tag="mm", bufs=4)
                        for kc in range(4):
                            nc.tensor.matmul(
                                psv[:rows, :],
                                vT16[:, kc * 2 + b,
                                     sub * 128:sub * 128 + rows],
                                wv[:, kc, :],
                                start=(kc == 0), stop=(kc == 3))
                        nc.vector.tensor_copy(vp[:rows, b, sub, :],
                                              psv[:rows, :])

            # ------------- phase B1: band attention (batched softmax) ------
            with tc.tile_pool(name="ps_s", bufs=2, space="PSUM") as ps_s, \
                 tc.tile_pool(name="ps_t", bufs=1, space="PSUM") as ps_tp, \
                 tc.tile_pool(name="ps_at", bufs=2, space="PSUM") as ps_at, \
                 tc.tile_pool(name="sat", bufs=2) as sat:
                for b in range(B):
                    for blk in range(2):
                        q0 = blk * 128
                        sc = sat.tile([128, 8, KSPAN], f32, tag="sc")
                        for h in range(H):
                            hp, ho = h // 2, (h % 2) * 64
                            pss = ps_s.tile([128, KSPAN], f32, tag="s")
                            nc.tensor.matmul(
                                pss[:],
                                qp[ho:ho + 64, hp, b, q0:q0 + 128],
                                kp[ho:ho + 64, b, q0:q0 + KSPAN],
                                start=True, stop=True)
                            nc.vector.tensor_add(sc[:, h, :], pss[:],
                                                 bias2[:, blk, :])
                        pr = sat.tile([128, 8, KSPAN], bf16, tag="pr")
                        nc.scalar.activation(pr[:], sc[:], AF.Exp,
                                             scale=0.125)
                        z = sat.tile([128, 8], f32, tag="z")
                        nc.vector.reduce_sum(z[:], pr[:], axis=X)
                        rz = sat.tile([128, 8], f32, tag="rz")
                        nc.vector.reciprocal(rz[:], z[:])
                        prn = sat.tile([128, 8, KSPAN], bf16, tag="prn")
                        nc.vector.tensor_mul(
                            prn[:], pr[:],
                            rz[:, :, None].broadcast_to([128, 8, KSPAN]))
                        pt = ps_tp.tile([128, 8, 2, 128], bf16, tag="tt")
                        for h in range(H):
                            nc.tensor.transpose(pt[:, h, 0, :],
                                                prn[:, h, 0:128], id16[:])
                            nc.tensor.transpose(pt[0:32, h, 1, :],
                                                prn[:, h, 128:KSPAN],
                                                id16[:])
                        pT = sat.tile([128, 8, 2, 128], bf16, tag="pT")
                        nc.vector.tensor_copy(pT[:, :, 0, :], pt[:, :, 0, :])
                        nc.scalar.copy(pT[0:32, :, 1, :], pt[0:32, :, 1, :])
                        pat = None
                        for h in range(H):
                            hp, ho = h // 2, (h % 2) * 64
                            if h % 2 == 0:
                                pat = ps_at.tile([128, 128], f32, tag="at")
                            nc.tensor.matmul(pat[ho:ho + 64, :],
                                             vp[:, b, blk, :], pT[:, h, 0, :],
                                             start=True, stop=False)
                            nc.tensor.matmul(pat[ho:ho + 64, :],
                                             vp[0:2 * W, b, blk + 1, :],
                                             pT[0:32, h, 1, :],
                                             start=False, stop=True)
                            if h % 2 == 1:
                                if hp % 2 == 0:
                                    nc.scalar.copy(aT[:, hp, b, blk, :],
                                                   pat[:])
                                else:
                                    nc.vector.tensor_copy(
                                        aT[:, hp, b, blk, :], pat[:])

            # ------------- phase B2: wo, gate, residual, rms ---------------
            with tc.tile_pool(name="pb2", bufs=3, space="PSUM") as pb2, \
                 tc.tile_pool(name="pbs", bufs=2, space="PSUM") as pbs, \
                 tc.tile_pool(name="pbt", bufs=2, space="PSUM") as pbt, \
                 tc.tile_pool(name="sb2", bufs=3) as sb2, \
                 tc.tile_pool(name="sxh", bufs=2) as sxh:
                for mt in range(4):
                    pwo = pb2.tile([128, 2, SH], f32, tag="mm")
                    for kc in range(4):
                        nc.tensor.matmul(
                            pwo[:], wo[:, kc, mt * 128:(mt + 1) * 128],
                            aT[:, kc, :, :, :],
                            start=(kc == 0), stop=(kc == 3))
                    nc.vector.tensor_scalar_add(attn32[:, mt, :, :], pwo[:],
                                                bo_t[:, mt:mt + 1])
                    nc.scalar.activation(attn16[:, mt, :, :], pwo[:],
                                         AF.Identity,
                                         bias=bo_t[:, mt:mt + 1])
                pss = None
                for mt in range(4):
                    pg = pb2.tile([128, 2, SH], f32, tag="mm")
                    for kc in range(8):
                        rhs = (qT16[:, (kc % 4) * 2:(kc % 4) * 2 + 2,
                                    1:SH + 1] if kc < 4
                               else attn16[:, kc - 4, :, :])
                        nc.tensor.matmul(
                            pg[:], wg[:, kc, mt * 128:(mt + 1) * 128],
                            rhs, start=(kc == 0), stop=(kc == 7))
                    gate = sb2.tile([128, 2, SH], f32, tag="gate")
                    nc.scalar.activation(gate[:], pg[:], AF.Silu,
                                         bias=bg_t[:, mt:mt + 1])
                    d1 = sb2.tile([128, 2, SH], f32, tag="d1")
                    nc.vector.tensor_sub(d1[:], qT32[:, mt * 2:mt * 2 + 2,
                                                     1:SH + 1],
                                         attn32[:, mt, :, :])
                    u = sb2.tile([128, 2, SH], f32, tag="u")
                    nc.vector.tensor_mul(u[:], gate[:], d1[:])
                    nc.vector.tensor_add(outr[:, mt, :, :],
                                         attn32[:, mt, :, :], u[:])
                    sq = sb2.tile([128, 2, SH], f32, tag="sq")
                    nc.scalar.square(sq[:], outr[:, mt, :, :])
                    if mt == 0:
                        pss = pbs.tile([1, 2, SH], f32, tag="ss", bufs=1)
                    nc.tensor.matmul(pss[:], ones_m1[:], sq[:],
                                     start=(mt == 0), stop=(mt == 3))
                sqr = sb2.tile([1, 2, SH], f32, tag="sqr")
                nc.scalar.activation(sqr[:], pss[:], AF.Sqrt,
                                     scale=1.0 / D, bias=cst[0:1, 0:1])
                rstd = sb2.tile([1, 2, SH], f32, tag="rstd")
                nc.vector.reciprocal(rstd[:], sqr[:])
                prb = pbs.tile([128, 2, SH], f32, tag="rb", bufs=1)
                nc.tensor.matmul(prb[:], ones_k1[:], rstd[:], start=True,
                                 stop=True)
                for mt in range(4):
                    nc.vector.scalar_tensor_tensor(
                        xh[:, mt, :, :], outr[:, mt, :, :],
                        rms_t[:, mt:mt + 1], prb[:],
                        op0=A.mult, op1=A.mult)
                for mt in range(4):
                    for b in range(B):
                        nc.vector.reduce_sum(
                            pc_sb[:, mt * 2 + b:mt * 2 + b + 1],
                            xh[:, mt, b, :], axis=X)
                for b in range(B):
                    for blk in range(2):
                        xt = sxh.tile([128, 512], f32, tag="xt")
                        for mt in range(4):
                            px = pbt.tile([128, 128], f32, tag="px")
                            nc.tensor.transpose(
                                px[:], xh[:, mt, b,
                                          blk * 128:blk * 128 + 128],
                                id32[:])
                            if mt % 2 == 0:
                                nc.scalar.copy(
                                    xt[:, mt * 128:(mt + 1) * 128], px[:])
                            else:
                                nc.vector.tensor_copy(
                                    xt[:, mt * 128:(mt + 1) * 128], px[:])
                        nc.sync.dma_start(
                            out_d[b, blk * 128:blk * 128 + 128, :], xt[:])
                nc.sync.dma_start(pc_d[:], pc_sb[:])

    nc.compile()
    return nc


def prep_inputs(inputs):
    """Full inputs dict -> list of 8 per-core in_maps."""
    f = lambda k: np.asarray(inputs[k], np.float32)
    query, key, value = f('query'), f('key'), f('value')

    qt = np.swapaxes(query, 1, 2)          # [B, D, S]
    kt = np.swapaxes(key, 1, 2)
    vt = np.swapaxes(value, 1, 2)
    qp1 = np.pad(qt, ((0, 0), (0, 0), (1, 1)))
    kpw = np.pad(kt, ((0, 0), (0, 0), (W, W)))
    vpw = np.pad(vt, ((0, 0), (0, 0), (W, W)))

    bo_eff = f('bo') + f('wo') @ np.tile(f('bv'), H)

    def pack(layout, parts, dtype):
        out = np.zeros((128, _cols(layout)), dtype)
        for n, c in layout:
            a = parts[n]
            out[:a.shape[0], _off(layout, n):_off(layout, n) + c] = a
        return out

    r = np.arange(128)
    j = np.arange(KSPAN)
    e16 = pack(EARLY16, {
        'wpe1T': _lhsT(f('wp_e1')), 'wpp1T': _lhsT(f('wp_p1')),
        'wpe2T': _lhsT(f('wp_e2')), 'wpp2T': _lhsT(f('wp_p2'))}, BF16)
    l16 = pack(LATE16, {
        'wqT': _lhsT(f('wq')), 'wkT': _lhsT(f('wk')), 'wvT': _lhsT(f('wv')),
        'woT': _lhsT(f('wo')), 'wgT': _lhsT(f('wg')),
        'ident16': np.eye(128, dtype=BF16)}, BF16)
    e32 = pack(EARLY32, {
        'wdw1': np.ascontiguousarray(
            f('wp_dw1').reshape(16, 128, 3).transpose(1, 0, 2)
        ).reshape(128, 48),
        'wdw2': _pcol(f('wp_dw2')[:, 0]),
        'cst': np.array([[1e-6, 3.0]], np.float32)}, np.float32)

    shared = {'early16': e16, 'late16': l16, 'early32': e32}
    l32_parts = {
        'ident32': np.eye(128, dtype=np.float32),
        'bq_t': _pcol(f('bq')),
        'bk_t': np.tile(f('bk'), 2).reshape(128, 1).astype(np.float32),
        'bo_t': _pcol(bo_eff), 'bg_t': _pcol(f('bg')),
        'rms_t': _pcol(f('rms_w')),
        'ones_k1': np.ones((1, 128), np.float32),
        'ones_m1': np.ones((128, 1), np.float32),
        'c2band': (2.0 * np.abs(j[None, :] - W - r[:, None])
                   ).astype(np.float32),
    }

    def tr8(x):  # [B, D, cols] -> [128, kc*2+b, cols]
        cols = x.shape[2]
        return np.ascontiguousarray(
            x.reshape(B, 4, 128, cols).transpose(2, 1, 0, 3)
            .reshape(128, 8, cols))

    maps = []
    for c in range(NCORES):
        s0 = c * SH
        m = dict(shared)
        m['qT32'] = tr8(qp1[:, :, s0:s0 + SH + 2])
        m['qT16'] = m['qT32'].astype(BF16)
        m['kT16'] = tr8(kpw[:, :, s0:s0 + SH + 2 * W]).astype(BF16)
        m['vT16'] = tr8(vpw[:, :, s0:s0 + SH + 2 * W]).astype(BF16)
        edge = np.zeros((128, 2 * KSPAN), np.float32)
        for blk in range(2):
            kidx = s0 + blk * 128 - W + j         # global key index per col
            edge[:, blk * KSPAN:(blk + 1) * KSPAN][
                :, (kidx < 0) | (kidx >= S)] = NEG
        m['late32'] = pack(LATE32, {**l32_parts, 'edge': edge}, np.float32)
        maps.append(m)
    return maps


def _get_program():
    if 'nc' not in _CACHE:
        _CACHE['nc'] = build_program()
    return _CACHE['nc']


def finish(results):
    """Gather per-core outputs -> full [B, S, D] (global seq-mean subtract)."""
    xh = np.concatenate([r['out_xh'] for r in results], axis=1)
    pc = np.zeros((128, 8), np.float64)
    for r in results:
        pc += r['pc']
    # pc[p, mt*2+b] -> batch b, feature o = mt*128+p
    pcv = pc.reshape(128, 4, 2).transpose(2, 1, 0).reshape(2, 512)
    mean = (pcv / S).astype(np.float32)
    return (xh - mean[:, None, :]).astype(np.float32)


def kernel(**inputs):
    from concourse.bass_utils import run_bass_kernel_spmd
    nc = _get_program()
    maps = prep_inputs(inputs)
    res = run_bass_kernel_spmd(nc, maps, list(range(NCORES)))
    return finish(res.results)


# revision 14
# speedup vs baseline: 2.0015x; 1.1010x over previous
"""Trainium2 Bass kernel for nn_EnhancedEncoderLayer (MQA sliding-window attention).

Strategy: sequence-parallel over S=2048 -> 8 cores x 256 rows (+halos).
Per core (all matmuls bf16 with f32 accumulate, elementwise f32):
  1. window-prediction net on its seq shard (InvRes(k3) -> gelu -> InvRes(k1)),
     local-shard mean as ratio estimate (the attention mask is a step function
     of ratio with enormous margin: any |err| < 1/29 keeps it identical, and
     local shard means agree with the global mean to ~1e-5).
  2. MQA projections in transposed [feat, seq] layout, 33-wide band attention
     computed as a dense [128q, 160k] block per 128-row query block with the
     runtime mask  allowed(d) <=> t >= 2|d|,  t = 3 + 29*ratio  (exactly
     reproduces the reference's -1e9 dense masking: exp underflows to 0).
     Softmax is max-free (scores/8 are O(1); exp cannot overflow) and batched
     across all 8 heads per query block.
  3. out-proj, SiLU gate, residual mix, RMS norm, per-core partial seq-sums.
     Batch pairs are merged into contiguous N=512 moving operands.
Host side: gathers shards and subtracts the global seq-mean (a [2,512] vector
computed from the per-core partial sums).  v-bias is folded into bo on host
(bo_eff = bo + wo @ tile(bv, H)).
"""
import numpy as np
import ml_dtypes

BF16 = ml_dtypes.bfloat16

B, S, D, H = 2, 2048, 512, 8
HD = D // H           # 64
NCORES = 8
SH = S // NCORES      # 256 rows per core
W = 16                # max band halfwidth (MAXW//2)
KSPAN = 160           # keys per 128-query block: 128 + 2*W
BN_S = float((1.0 + 1e-5) ** -0.5)
NEG = -1e9

# merged weight slab layouts: name -> cols
EARLY16 = [('wpe2T', 512), ('wpp2T', 4)]
LATE16 = [('wqT', 4 * 512), ('wkT', 4 * 64), ('wvT', 4 * 64),
          ('woT', 4 * 512), ('wgT', 8 * 512), ('ident16', 128),
          ('ones16', 1)]
EARLY32 = [('wdw1', 48), ('wdw2', 4), ('cst', 2)]
LATE32 = [('bq_t', 4), ('bk_t', 1), ('bo_t', 4),
          ('bg_t', 4), ('rms_t', 4), ('ones_k1', 128),
          ('c2band', KSPAN), ('edge', 2 * KSPAN)]


def _cols(layout):
    return sum(c for _, c in layout)


def _off(layout, name):
    o = 0
    for n, c in layout:
        if n == name:
            return o
        o += c
    raise KeyError(name)


_CACHE = {}


def _lhsT(w):
    # w [O, C] -> stationary-operand slab [128, C//128 * O] bf16
    C = w.shape[1]
    return np.ascontiguousarray(
        w.T.reshape(C // 128, 128, w.shape[0]).transpose(1, 0, 2)
        .reshape(128, -1).astype(BF16))


def _pcol(v):
    # per-feature vector [D] -> per-partition layout [128, D//128]
    return np.ascontiguousarray(v.reshape(-1, 128).T.astype(np.float32))


def build_program():
    import concourse.bacc as bacc
    import concourse.mybir as mybir
    from concourse.tile import TileContext

    dt = mybir.dt
    f32, bf16 = dt.float32, dt.bfloat16
    A = mybir.AluOpType
    AF = mybir.ActivationFunctionType
    X = mybir.AxisListType.X
    XY = mybir.AxisListType.XY

    nc = bacc.Bacc("TRN2", target_bir_lowering=False, debug=False,
                   num_devices=NCORES)

    di = lambda n, s, d=f32: nc.dram_tensor(n, s, d, kind="ExternalInput")
    # activation tensors, host layout [128, kc*2+b, s] (kc-major, b-minor)
    qT32_d = di("qT32", [128, 8, SH + 2])
    qT16_d = di("qT16", [128, 8, SH + 2], bf16)
    kT16_d = di("kT16", [128, 8, SH + 2 * W], bf16)
    vT16_d = di("vT16", [128, 8, SH + 2 * W], bf16)
    fp8 = dt.float8e4
    qT8_d = di("qT8", [128, 2, 2, 2, 272], fp8)
    wpe18_d = di("wpe18", [128, 2, 2, 2048], fp8)
    wpp18_d = di("wpp18", [128, 8, 2, 128], fp8)
    e16_d = di("early16", [128, _cols(EARLY16)], bf16)
    l16_d = di("late16", [128, _cols(LATE16)], bf16)
    e32_d = di("early32", [128, _cols(EARLY32)])
    l32_d = di("late32", [128, _cols(LATE32)])

    out_d = nc.dram_tensor("out_xh", [B, D, SH], f32, kind="ExternalOutput")
    pc_d = nc.dram_tensor("pc", [128, 8], f32, kind="ExternalOutput")

    with TileContext(nc) as tc:
        with tc.tile_pool(name="const", bufs=1) as cpool:
            # ordered loads: stage-1 critical first, everything on sync
            qT8 = cpool.tile([128, 2, 2, 2, 272], fp8, tag="qT8s")
            nc.sync.dma_start(qT8[:], qT8_d[:])
            wpe18 = cpool.tile([128, 2, 2, 2048], fp8, tag="wpe18")
            nc.sync.dma_start(wpe18[:], wpe18_d[:])
            wpp18 = cpool.tile([128, 8, 2, 128], fp8, tag="wpp18")
            nc.sync.dma_start(wpp18[:], wpp18_d[:])
            qT16 = cpool.tile([128, 8, SH + 2], bf16, tag="qT16s")
            nc.sync.dma_start(qT16[:], qT16_d[:])
            e16 = cpool.tile([128, _cols(EARLY16)], bf16, tag="e16")
            nc.sync.dma_start(e16[:], e16_d[:])
            e32 = cpool.tile([128, _cols(EARLY32)], f32, tag="e32")
            nc.sync.dma_start(e32[:], e32_d[:])
            kT16 = cpool.tile([128, 8, SH + 2 * W], bf16, tag="kT16s")
            nc.sync.dma_start(kT16[:], kT16_d[:])
            vT16 = cpool.tile([128, 8, SH + 2 * W], bf16, tag="vT16s")
            nc.sync.dma_start(vT16[:], vT16_d[:])
            l16 = cpool.tile([128, _cols(LATE16)], bf16, tag="l16")
            nc.sync.dma_start(l16[:], l16_d[:])
            l32 = cpool.tile([128, _cols(LATE32)], f32, tag="l32")
            nc.sync.dma_start(l32[:], l32_d[:])
            qT32 = cpool.tile([128, 8, SH + 2], f32, tag="qT32s")
            nc.sync.dma_start(qT32[:], qT32_d[:])

            # weight slab views
            wpe2 = e16[:, 0:512]
            wpp2 = e16[:, 512:516].rearrange("p (kc m) -> p kc m", kc=4)
            lo = lambda n: _off(LATE16, n)
            wq = l16[:, lo('wqT'):lo('wqT') + 2048].rearrange(
                "p (kc m) -> p kc m", kc=4)
            wk = l16[:, lo('wkT'):lo('wkT') + 256].rearrange(
                "p (kc m) -> p kc m", kc=4)
            wv = l16[:, lo('wvT'):lo('wvT') + 256].rearrange(
                "p (kc m) -> p kc m", kc=4)
            wo = l16[:, lo('woT'):lo('woT') + 2048].rearrange(
                "p (kc m) -> p kc m", kc=4)
            wg = l16[:, lo('wgT'):lo('wgT') + 4096].rearrange(
                "p (kc m) -> p kc m", kc=8)
            id16 = l16[:, lo('ident16'):lo('ident16') + 128]
            ones16 = l16[:, lo('ones16'):lo('ones16') + 1]
            wdw1 = e32[:, 0:48].rearrange("p (mt t) -> p mt t", mt=16)
            wdw2 = e32[:, 48:52]
            cst = e32[0:1, 52:54]
            go = lambda n: _off(LATE32, n)
            bq_t = l32[:, go('bq_t'):go('bq_t') + 4]
            bk_t = l32[:, go('bk_t'):go('bk_t') + 1]
            bo_t = l32[:, go('bo_t'):go('bo_t') + 4]
            bg_t = l32[:, go('bg_t'):go('bg_t') + 4]
            rms_t = l32[:, go('rms_t'):go('rms_t') + 4]
            ones_k1 = l32[0:1, go('ones_k1'):go('ones_k1') + 128]
            c2 = l32[:, go('c2band'):go('c2band') + KSPAN]
            edge = l32[:, go('edge'):go('edge') + 2 * KSPAN].rearrange(
                "p (blk j) -> p blk j", blk=2)

            # resident intermediates (batch-pair contiguous layouts)
            hdw = cpool.tile([128, 16, 2, SH], fp8, tag="hdw")   # (mt, b)
            g1 = cpool.tile([128, 2, SH], bf16, tag="g1")
            qp = cpool.tile([128, 4, 2, SH], bf16, tag="qp")      # (hp, b)
            kp = cpool.tile([128, 2, SH + 2 * W], bf16, tag="kp")
            vp = cpool.tile([128, 2, 3, HD], bf16, tag="vp")
            aT = cpool.tile([128, 4, 2, 2, 128], bf16, tag="aT")  # (hp,b,blk)
            attn32 = cpool.tile([128, 4, 2, SH], f32, tag="attn32")
            attn16 = cpool.tile([128, 4, 2, SH], bf16, tag="attn16")
            outr = cpool.tile([128, 4, 2, SH], f32, tag="outr")
            xh = cpool.tile([128, 4, 2, SH], f32, tag="xh")
            pc_sb = cpool.tile([128, 8], f32, tag="pc")
            bias2 = cpool.tile([128, 2, KSPAN], f32, tag="bias2")
            t128 = cpool.tile([128, 1], f32, tag="t128")

            # ------------- phase A: prediction net + qkv projections -------
            with tc.tile_pool(name="pa", bufs=6, space="PSUM") as pa, \
                 tc.tile_pool(name="sa", bufs=3) as sa, \
                 tc.tile_pool(name="sb1", bufs=3) as sb1:

                # inverted residual 1 (expand 512->2048, dw k3, proj -> 128)
                DR = mybir.MatmulPerfMode.DoubleRow
                for mt in range(16):
                    for b in range(B):
                        ps = pa.tile([128, SH + 2], f32, tag="mm", bufs=4)
                        for kc2 in range(2):
                            nc.tensor.matmul(
                                ps[:],
                                wpe18[:, kc2, :, mt * 128:(mt + 1) * 128],
                                qT8[:, kc2, b, :, 0:SH + 2],
                                start=(kc2 == 0), stop=(kc2 == 1),
                                perf_mode=DR)
                        h1 = sa.tile([128, SH + 2], bf16, tag="h1")
                        nc.scalar.activation(h1[:], ps[:], AF.Silu,
                                             scale=1.0 / 64)
                        t1 = sa.tile([128, SH], bf16, tag="t1")
                        nc.vector.tensor_scalar_mul(
                            t1[:], h1[:, 2:SH + 2], wdw1[:, mt, 2:3])
                        t2 = sa.tile([128, SH], bf16, tag="t2")
                        nc.vector.scalar_tensor_tensor(
                            t2[:], h1[:, 1:SH + 1], wdw1[:, mt, 1:2], t1[:],
                            op0=A.mult, op1=A.add)
                        t3 = sa.tile([128, SH], bf16, tag="t3")
                        nc.vector.scalar_tensor_tensor(
                            t3[:], h1[:, 0:SH], wdw1[:, mt, 0:1], t2[:],
                            op0=A.mult, op1=A.add)
                        nc.scalar.activation(hdw[:, mt, b, :], t3[:],
                                             AF.Silu, scale=BN_S)
                # project 2048 -> 128 (both batches, N=512), * BN_S, gelu
                ps2 = pa.tile([128, 2, SH], f32, tag="mm", bufs=4)
                for b in range(B):
                    for kc2 in range(8):
                        nc.tensor.matmul(
                            ps2[:, b, :], wpp18[:, kc2, :, :],
                            hdw[:, 2 * kc2:2 * kc2 + 2, b, :],
                            start=(kc2 == 0), stop=(kc2 == 7),
                            perf_mode=DR)
                nc.scalar.activation(g1[:], ps2[:], AF.Gelu,
                                     scale=BN_S / 64)

                # inverted residual 2 (128->512, dw k1, proj -> 1) + ratio
                ps4 = pa.tile([1, 2, SH], f32, tag="p4", bufs=1)
                for mt in range(4):
                    ps3 = pa.tile([128, 2, SH], f32, tag="mm", bufs=4)
                    nc.tensor.matmul(ps3[:],
                                     wpe2[:, mt * 128:(mt + 1) * 128],
                                     g1[:], start=True, stop=True)
                    s1 = sb1.tile([128, 2, SH], bf16, tag="s1")
                    nc.scalar.activation(s1[:], ps3[:], AF.Silu)
                    s2 = sb1.tile([128, 2, SH], bf16, tag="s2")
                    nc.vector.tensor_scalar_mul(s2[:], s1[:],
                                                wdw2[:, mt:mt + 1])
                    s3 = sb1.tile([128, 2, SH], bf16, tag="s3")
                    nc.scalar.activation(s3[:], s2[:], AF.Silu, scale=BN_S)
                    nc.tensor.matmul(ps4[:], wpp2[:, mt, :], s3[:],
                                     start=(mt == 0), stop=(mt == 3))
                r0 = sb1.tile([1, 1], f32, tag="r0")
                nc.vector.reduce_sum(r0[:], ps4[:], axis=XY)
                t_sc = sb1.tile([1, 1], f32, tag="tsc")
                nc.scalar.activation(t_sc[:], r0[:], AF.Identity,
                                     scale=29.0 * BN_S / (B * SH),
                                     bias=cst[0:1, 1:2])
                ps_t = pa.tile([128, 1], f32, tag="mm", bufs=4)
                nc.tensor.matmul(ps_t[:], ones_k1[:], t_sc[:], start=True,
                                 stop=True)
                nc.scalar.copy(t128[:], ps_t[:])
                mask = sb1.tile([128, KSPAN], f32, tag="mask")
                nc.vector.tensor_scalar(mask[:], c2[:], t128[:, 0:1], None,
                                        op0=A.is_le)
                bb = sb1.tile([128, KSPAN], f32, tag="bb")
                nc.scalar.activation(bb[:], mask[:], AF.Copy, scale=1e9,
                                     bias=-1e9)
                for blk in range(2):
                    nc.vector.tensor_add(bias2[:, blk, :], bb[:],
                                         edge[:, blk, :])

                # q/k/v projections
                for mt in range(4):
                    psq = pa.tile([128, 2, SH], f32, tag="mm", bufs=4)
                    for kc in range(4):
                        nc.tensor.matmul(
                            psq[:], wq[:, kc, mt * 128:(mt + 1) * 128],
                            qT16[:, kc * 2:kc * 2 + 2, 1:SH + 1],
                            start=(kc == 0), stop=(kc == 3))
                    nc.scalar.activation(qp[:, mt, :, :], psq[:], AF.Identity,
                                         bias=bq_t[:, mt:mt + 1])
                for b in range(B):
                    psk = pa.tile([64, SH + 2 * W], f32, tag="kpp", bufs=1)
                    for kc in range(4):
                        nc.tensor.matmul(psk[:], wk[:, kc, :],
                                         kT16[:, kc * 2 + b, :],
                                         start=(kc == 0), stop=(kc == 3))
                    nc.scalar.activation(kp[0:64, b, :], psk[:], AF.Identity,
                                         bias=bk_t[0:64, 0:1])
                    nc.sync.dma_start(kp[64:128, b, :], kp[0:64, b, :])
                    for sub in range(3):
                        rows = 128 if sub < 2 else 2 * W
                        psv = pa.tile([128, HD], f32, tag="mm", bufs=4)
                        for kc in range(4):
                            nc.tensor.matmul(
                                psv[:rows, :],
                                vT16[:, kc * 2 + b,
                                     sub * 128:sub * 128 + rows],
                                wv[:, kc, :],
                                start=(kc == 0), stop=(kc == 3))
                        nc.vector.tensor_copy(vp[:rows, b, sub, :],
                                              psv[:rows, :])

            # ------------- phase B1: band attention (batched softmax) ------
            with tc.tile_pool(name="ps_s", bufs=2, space="PSUM") as ps_s, \
                 tc.tile_pool(name="ps_t", bufs=2, space="PSUM") as ps_tp, \
                 tc.tile_pool(name="ps_at", bufs=2, space="PSUM") as ps_at, \
                 tc.tile_pool(name="sat", bufs=2) as sat:
                for b in range(B):
                    for blk in range(2):
                        q0 = blk * 128
                        sc = sat.tile([128, 8, KSPAN], f32, tag="sc")
                        for h in range(H):
                            hp, ho = h // 2, (h % 2) * 64
                            pss = ps_s.tile([128, KSPAN], f32, tag="s")
                            nc.tensor.matmul(
                                pss[:],
                                qp[ho:ho + 64, hp, b, q0:q0 + 128],
                                kp[ho:ho + 64, b, q0:q0 + KSPAN],
                                start=True, stop=True)
                            nc.vector.tensor_add(sc[:, h, :], pss[:],
                                                 bias2[:, blk, :])
                        pr = sat.tile([128, 8, KSPAN], bf16, tag="pr")
                        nc.scalar.activation(pr[:], sc[:], AF.Exp,
                                             scale=0.125)
                        z = sat.tile([128, 8], f32, tag="z")
                        nc.vector.reduce_sum(z[:], pr[:], axis=X)
                        rz = sat.tile([128, 8], f32, tag="rz")
                        nc.vector.reciprocal(rz[:], z[:])
                        prn = sat.tile([128, 8, KSPAN], bf16, tag="prn")
                        nc.vector.tensor_mul(
                            prn[:], pr[:],
                            rz[:, :, None].broadcast_to([128, 8, KSPAN]))
                        pt = ps_tp.tile([128, 8, 2, 128], bf16, tag="tt")
                        for h in range(H):
                            nc.tensor.transpose(pt[:, h, 0, :],
                                                prn[:, h, 0:128], id16[:])
                            nc.tensor.transpose(pt[0:32, h, 1, :],
                                                prn[:, h, 128:KSPAN],
                                                id16[:])
                        pT = sat.tile([128, 8, 2, 128], bf16, tag="pT")
                        nc.vector.tensor_copy(pT[:, :, 0, :], pt[:, :, 0, :])
                        nc.scalar.copy(pT[0:32, :, 1, :], pt[0:32, :, 1, :])
                        pat = None
                        for h in range(H):
                            hp, ho = h // 2, (h % 2) * 64
                            if h % 2 == 0:
                                pat = ps_at.tile([128, 128], f32, tag="at")
                            nc.tensor.matmul(pat[ho:ho + 64, :],
                                             vp[:, b, blk, :], pT[:, h, 0, :],
                                             start=True, stop=False)
                            nc.tensor.matmul(pat[ho:ho + 64, :],
                                             vp[0:2 * W, b, blk + 1, :],
                                             pT[0:32, h, 1, :],
                                             start=False, stop=True)
                            if h % 2 == 1:
                                if hp % 2 == 0:
                                    nc.scalar.copy(aT[:, hp, b, blk, :],
                                                   pat[:])
                                else:
                                    nc.vector.tensor_copy(
                                        aT[:, hp, b, blk, :], pat[:])

            # ------------- phase B2: wo, gate, residual, rms ---------------
            with tc.tile_pool(name="pb2", bufs=3, space="PSUM") as pb2, \
                 tc.tile_pool(name="pbs", bufs=2, space="PSUM") as pbs, \
                 tc.tile_pool(name="sb2", bufs=3) as sb2:
                for mt in range(4):
                    pwo = pb2.tile([128, 2, SH], f32, tag="mm")
                    for kc in range(4):
                        nc.tensor.matmul(
                            pwo[:], wo[:, kc, mt * 128:(mt + 1) * 128],
                            aT[:, kc, :, :, :],
                            start=(kc == 0), stop=(kc == 3))
                    nc.vector.tensor_scalar_add(attn32[:, mt, :, :], pwo[:],
                                                bo_t[:, mt:mt + 1])
                    nc.scalar.activation(attn16[:, mt, :, :], pwo[:],
                                         AF.Identity,
                                         bias=bo_t[:, mt:mt + 1])
                pss = None
                for mt in range(4):
                    pg = pb2.tile([128, 2, SH], f32, tag="mm")
                    for kc in range(8):
                        rhs = (qT16[:, (kc % 4) * 2:(kc % 4) * 2 + 2,
                                    1:SH + 1] if kc < 4
                               else attn16[:, kc - 4, :, :])
                        nc.tensor.matmul(
                            pg[:], wg[:, kc, mt * 128:(mt + 1) * 128],
                            rhs, start=(kc == 0), stop=(kc == 7))
                    gate = sb2.tile([128, 2, SH], f32, tag="gate")
                    nc.scalar.activation(gate[:], pg[:], AF.Silu,
                                         bias=bg_t[:, mt:mt + 1])
                    d1 = sb2.tile([128, 2, SH], f32, tag="d1")
                    nc.vector.tensor_sub(d1[:], qT32[:, mt * 2:mt * 2 + 2,
                                                     1:SH + 1],
                                         attn32[:, mt, :, :])
                    u = sb2.tile([128, 2, SH], f32, tag="u")
                    nc.vector.tensor_mul(u[:], gate[:], d1[:])
                    nc.vector.tensor_add(outr[:, mt, :, :],
                                         attn32[:, mt, :, :], u[:])
                    sq = sb2.tile([128, 2, SH], bf16, tag="sq")
                    nc.scalar.square(sq[:], outr[:, mt, :, :])
                    if mt == 0:
                        pss = pbs.tile([1, 2, SH], f32, tag="ss", bufs=1)
                    nc.tensor.matmul(pss[:], ones16[:], sq[:],
                                     start=(mt == 0), stop=(mt == 3))
                sqr = sb2.tile([1, 2, SH], f32, tag="sqr")
                nc.scalar.activation(sqr[:], pss[:], AF.Sqrt,
                                     scale=1.0 / D, bias=cst[0:1, 0:1])
                rstd = sb2.tile([1, 2, SH], f32, tag="rstd")
                nc.vector.reciprocal(rstd[:], sqr[:])
                prb = pbs.tile([128, 2, SH], f32, tag="rb", bufs=1)
                nc.tensor.matmul(prb[:], ones_k1[:], rstd[:], start=True,
                                 stop=True)
                for mt in range(4):
                    nc.vector.scalar_tensor_tensor(
                        xh[:, mt, :, :], outr[:, mt, :, :],
                        rms_t[:, mt:mt + 1], prb[:],
                        op0=A.mult, op1=A.mult)
                for mt in range(4):
                    for b in range(B):
                        nc.vector.reduce_sum(
                            pc_sb[:, mt * 2 + b:mt * 2 + b + 1],
                            xh[:, mt, b, :], axis=X)
                for mt in range(4):
                    for b in range(B):
                        nc.sync.dma_start(
                            out_d[b, mt * 128:(mt + 1) * 128, :],
                            xh[:, mt, b, :])
                nc.sync.dma_start(pc_d[:], pc_sb[:])

    nc.compile()
    return nc


def prep_inputs(inputs):
    """Full inputs dict -> list of 8 per-core in_maps."""
    f = lambda k: np.asarray(inputs[k], np.float32)
    query, key, value = f('query'), f('key'), f('value')

    qt = np.swapaxes(query, 1, 2)          # [B, D, S]
    kt = np.swapaxes(key, 1, 2)
    vt = np.swapaxes(value, 1, 2)
    qp1 = np.pad(qt, ((0, 0), (0, 0), (1, 1)))
    kpw = np.pad(kt, ((0, 0), (0, 0), (W, W)))
    vpw = np.pad(vt, ((0, 0), (0, 0), (W, W)))

    bo_eff = f('bo') + f('wo') @ np.tile(f('bv'), H)

    def pack(layout, parts, dtype):
        out = np.zeros((128, _cols(layout)), dtype)
        for n, c in layout:
            a = parts[n]
            out[:a.shape[0], _off(layout, n):_off(layout, n) + c] = a
        return out

    r = np.arange(128)
    j = np.arange(KSPAN)
    F8 = ml_dtypes.float8_e4m3
    w1 = (f('wp_e1').T * 64.0)      # [512, 2048]
    wpe18 = np.ascontiguousarray(
        w1.reshape(2, 2, 128, 2048).transpose(2, 0, 1, 3)).astype(F8)
    w2 = (f('wp_p1').T * 64.0)      # [2048, 128]
    wpp18 = np.ascontiguousarray(
        w2.reshape(8, 2, 128, 128).transpose(2, 0, 1, 3)).astype(F8)
    e16 = pack(EARLY16, {
        'wpe2T': _lhsT(f('wp_e2')), 'wpp2T': _lhsT(f('wp_p2'))}, BF16)
    l16 = pack(LATE16, {
        'wqT': _lhsT(f('wq')), 'wkT': _lhsT(f('wk')), 'wvT': _lhsT(f('wv')),
        'woT': _lhsT(f('wo')), 'wgT': _lhsT(f('wg')),
        'ident16': np.eye(128, dtype=BF16),
        'ones16': np.ones((128, 1), BF16)}, BF16)
    e32 = pack(EARLY32, {
        'wdw1': np.ascontiguousarray(
            f('wp_dw1').reshape(16, 128, 3).transpose(1, 0, 2)
        ).reshape(128, 48),
        'wdw2': _pcol(f('wp_dw2')[:, 0]),
        'cst': np.array([[1e-6, 3.0]], np.float32)}, np.float32)

    shared = {'early16': e16, 'late16': l16, 'early32': e32,
              'wpe18': wpe18, 'wpp18': wpp18}
    l32_parts = {
        'bq_t': _pcol(f('bq')),
        'bk_t': np.tile(f('bk'), 2).reshape(128, 1).astype(np.float32),
        'bo_t': _pcol(bo_eff), 'bg_t': _pcol(f('bg')),
        'rms_t': _pcol(f('rms_w')),
        'ones_k1': np.ones((1, 128), np.float32),
        'c2band': (2.0 * np.abs(j[None, :] - W - r[:, None])
                   ).astype(np.float32),
    }

    def tr8(x):  # [B, D, cols] -> [128, kc*2+b, cols]
        cols = x.shape[2]
        return np.ascontiguousarray(
            x.reshape(B, 4, 128, cols).transpose(2, 1, 0, 3)
            .reshape(128, 8, cols))

    maps = []
    for c in range(NCORES):
        s0 = c * SH
        m = dict(shared)
        m['qT32'] = tr8(qp1[:, :, s0:s0 + SH + 2])
        m['qT16'] = m['qT32'].astype(BF16)
        q8 = np.zeros((128, 2, 2, 2, 272), F8)
        q8[:, :, :, :, 0:SH + 2] = (
            qp1[:, :, s0:s0 + SH + 2]
            .reshape(2, 2, 2, 128, SH + 2).transpose(3, 1, 0, 2, 4)
            .astype(F8))
        m['qT8'] = q8
        m['kT16'] = tr8(kpw[:, :, s0:s0 + SH + 2 * W]).astype(BF16)
        m['vT16'] = tr8(vpw[:, :, s0:s0 + SH + 2 * W]).astype(BF16)
        edge = np.zeros((128, 2 * KSPAN), np.float32)
        for blk in range(2):
            kidx = s0 + blk * 128 - W + j         # global key index per col
            edge[:, blk * KSPAN:(blk + 1) * KSPAN][
                :, (kidx < 0) | (kidx >= S)] = NEG
        m['late32'] = pack(LATE32, {**l32_parts, 'edge': edge}, np.float32)
        maps.append(m)
    return maps


def _get_program():
    if 'nc' not in _CACHE:
        _CACHE['nc'] = build_program()
    return _CACHE['nc']


def finish(results):
    """Gather per-core outputs -> full [B, S, D] (global seq-mean subtract)."""
    xh = np.concatenate([r['out_xh'] for r in results], axis=2)  # [B,D,S]
    pc = np.zeros((128, 8), np.float64)
    for r in results:
        pc += r['pc']
    # pc[p, mt*2+b] -> batch b, feature o = mt*128+p
    pcv = pc.reshape(128, 4, 2).transpose(2, 1, 0).reshape(2, 512)
    mean = (pcv / S).astype(np.float32)
    out = xh - mean[:, :, None]
    return np.ascontiguousarray(out.transpose(0, 2, 1)).astype(np.float32)


def kernel(**inputs):
    from concourse.bass_utils import run_bass_kernel_spmd
    nc = _get_program()
    maps = prep_inputs(inputs)
    res = run_bass_kernel_spmd(nc, maps, list(range(NCORES)))
    return finish(res.results)
